# revision 15
# baseline (speedup 1.0000x reference)
"""Self-contained Trainium2 Bass kernel for 4-layer GraphSAGE (nn_LASAGE).

Strategy (v2):
  - Nodes dst-sharded across 8 cores (6250/core, padded to 6272 = 49 blocks of 128).
  - Aggregation POST-matmul: agg(x)@Wl == agg(x@Wl). Per layer each core
    computes y = h@Wl and z = (h@Wr + b)*deg for its own shard in one fused
    per-block pass, an AllGather replicates the bf16 Y table to every core's
    DRAM, and edges gather y[src] rows with dma_gather (int16 idx, table split
    in two halves so indices fit int16).
  - Big gather calls (up to 36 tiles = 4608 idxs) spanning dst blocks amortize
    the ~1us SWDGE fixed overhead per call.
  - Scatter-add via one-hot matmuls: psum[dst, f] += onehot[e, dst].T @ g[e, f]
    with the UNWEIGHTED one-hot (iota == dstcol) as the stationary operand and
    the 256-wide gathered rows moving -> one matmul per edge tile.
  - Mean normalization folds into the epilogue: out = relu((agg + z*deg)*invd)
    via scalar-engine activation with per-partition scale, where z*deg is
    injected into the psum by an identity matmul.
  - Two phases per layer (src half 0, then 1) with a bf16 SBUF accumulator in
    between, so the second half-table AllGather hides behind phase A.
  - Final layer's table rows pack [y3 | z3] into 128 bf16 cols (256B rows, the
    dma_gather minimum) so the output scatter also runs in bf16.
"""
import sys, os, types

sys.path.insert(0, "/opt/trn_rl_repo")
import numpy as np

N = 50000
E = 800000
NCORES = 8
S = N // NCORES            # 6250 real nodes per core
SP = 6272                  # padded (49 blocks of 128)
NBLK = SP // 128
SPH = SP // 2              # 3136: local-row split for the two AG half-tables
HALF = NCORES * SPH        # 25088 rows per half-table (int16-safe)
D1 = 256                   # concat(h0, h1)
DM = 256
DO = 64
TPC = 8                    # max tiles (of 128 idxs) per dma_gather call
                           # (HW-probed: 1024 idxs OK, 1280+ crashes the device
                           # -- SWDGE descriptor-ring capacity per queue)


def _install_hooks():
    """antenv.axon_hooks shim so trace=True works in this image (optional)."""
    try:
        import antenv
        if "antenv.axon_hooks" not in sys.modules:
            mod = types.ModuleType("antenv.axon_hooks")
            mod._hook = None
            mod.set_axon_ntff_profile_hook = lambda h: setattr(mod, "_hook", h)
            mod.get_axon_ntff_profile_hook = lambda: mod._hook
            sys.modules["antenv.axon_hooks"] = mod
            antenv.axon_hooks = mod
        from antenv.axon_hooks import get_axon_ntff_profile_hook, set_axon_ntff_profile_hook
        if get_axon_ntff_profile_hook() is None:
            from trn_agent_boot.trn_boot import _ntff_profile_via_ctypes
            set_axon_ntff_profile_hook(_ntff_profile_via_ctypes("/opt/axon/libaxon_pjrt.so"))
        import concourse.bass_utils as bu
        bu.upload_artifacts = lambda tmpdir: f"file://{tmpdir}"
    except Exception:
        pass


def _preprocess(edge_index):
    """Edge lists per core, grouped by (src half, dst block), padded per-tile."""
    src = np.asarray(edge_index[0], np.int64)
    dst = np.asarray(edge_index[1], np.int64)
    core = dst // S
    dl = (dst % S).astype(np.int64)
    blk = dl // 128
    col = dl % 128
    sloc = src % S
    half = (sloc >= SPH).astype(np.int64)
    grow = (src // S) * SPH + (sloc - half * SPH)   # row within its half-table

    deg = np.bincount(core * S + dl, minlength=N).reshape(NCORES, S)

    order = np.lexsort((grow, blk, half, core))
    core_s, half_s, blk_s, col_s, row_s = (core[order], half[order], blk[order],
                                           col[order], grow[order])

    key = (core_s * 2 + half_s) * NBLK + blk_s
    counts = np.bincount(key, minlength=NCORES * 2 * NBLK).reshape(NCORES, 2, NBLK)
    tiles_hb = np.ceil(counts.max(axis=0) / 128).astype(np.int64)   # [2, NBLK]
    tiles_hb = np.maximum(tiles_hb, 1)

    pad_hb = tiles_hb * 128
    tot_h = pad_hb.sum(axis=1)
    seg_off = np.zeros((2, NBLK), np.int64)
    seg_off[:, 1:] = np.cumsum(pad_hb, axis=1)[:, :-1]

    srcpad = np.zeros((NCORES, 2), dtype=object)
    colpad = np.zeros((NCORES, 2), dtype=object)
    for c in range(NCORES):
        for h in range(2):
            srcpad[c, h] = np.zeros(int(tot_h[h]), np.int64)
            colpad[c, h] = np.full(int(tot_h[h]), -1, np.int64)
    grp = key
    first = np.r_[True, grp[1:] != grp[:-1]]
    gidx = np.arange(len(grp)) - np.maximum.accumulate(np.where(first, np.arange(len(grp)), 0))
    pos = seg_off[half_s, blk_s] + gidx
    for c in range(NCORES):
        m = core_s == c
        for h in range(2):
            mh = m & (half_s == h)
            p = pos[mh]
            srcpad[c, h][p] = row_s[mh]
            colpad[c, h][p] = col_s[mh]

    return {
        "tiles_hb": tiles_hb, "seg_off": seg_off,
        "srcpad": srcpad, "colpad": colpad, "deg": deg,
    }


def _build_callplan(tiles_hb):
    """Gather call plan: per half, chunks of up to TPC tiles spanning blocks.

    Returns calls (list of dicts) and block_tiles[h][b] = [(ci, slot, dcol)].
    dcol is the global tile column (h-major) into the dstl image.
    """
    T = [int(tiles_hb[0].sum()), int(tiles_hb[1].sum())]
    cum = np.zeros((2, NBLK), np.int64)
    cum[:, 1:] = np.cumsum(tiles_hb, axis=1)[:, :-1]
    calls = []
    call_base = [0, 0]
    for h in range(2):
        call_base[h] = len(calls)
        t = 0
        while t < T[h]:
            k = min(TPC, T[h] - t)
            calls.append(dict(h=h, t0=t, k=k))
            t += k
    block_tiles = [[[] for _ in range(NBLK)] for _ in range(2)]
    for h in range(2):
        off = 0 if h == 0 else T[0]
        for b in range(NBLK):
            for t in range(int(cum[h, b]), int(cum[h, b] + tiles_hb[h, b])):
                ci = call_base[h] + t // TPC
                slot = t % TPC
                block_tiles[h][b].append((ci, slot, off + t))
    return calls, block_tiles, T


def _idx_arrays(pre, core):
    """int16 idx image [128, (T0+T1)*8] and dst-col image [128, T0+T1] bf16."""
    import ml_dtypes as _ml
    imgs, cols = [], []
    for h in range(2):
        sp = pre["srcpad"][core, h]
        cp = pre["colpad"][core, h]
        imgs.append(sp.reshape(-1, 16).T.astype(np.int16))    # [16, tot/16]
        cols.append(cp.reshape(-1, 128).T.astype(np.float32)) # [128, T_h]
    idx_img = np.hstack(imgs)
    dstl = np.hstack(cols).astype(_ml.bfloat16)
    return np.tile(idx_img, (8, 1)), dstl


def _build_bass(pre, calls, block_tiles, T):
    import concourse.bass as bass
    import concourse.bacc as bacc
    import concourse.mybir as mybir
    import concourse.tile as tile

    FP32 = mybir.dt.float32
    BF16 = mybir.dt.bfloat16
    I16 = mybir.dt.int16
    AL = mybir.AluOpType
    AF = mybir.ActivationFunctionType

    TT = T[0] + T[1]
    IDXC = TT * 8

    nc = bacc.Bacc("TRN2", target_bir_lowering=False, debug=False,
                   enable_asserts=False, num_devices=NCORES, num_swdge_queues=4)

    x0T = nc.dram_tensor("x0T", [128, SP], BF16, kind="ExternalInput")
    x1T = nc.dram_tensor("x1T", [128, SP], BF16, kind="ExternalInput")
    wlr0 = nc.dram_tensor("wlr0", [128, 256], BF16, kind="ExternalInput")
    wlr1 = nc.dram_tensor("wlr1", [128, 256], BF16, kind="ExternalInput")
    wlrm = nc.dram_tensor("wlrm", [256, 512], BF16, kind="ExternalInput")
    wlro = nc.dram_tensor("wlro", [256, 128], BF16, kind="ExternalInput")
    b01_0d = nc.dram_tensor("b01_0", [1, 256], BF16, kind="ExternalInput")
    b01_1d = nc.dram_tensor("b01_1", [1, 256], BF16, kind="ExternalInput")
    bmd = nc.dram_tensor("bmc", [1, 512], BF16, kind="ExternalInput")
    bod = nc.dram_tensor("boc", [1, 128], BF16, kind="ExternalInput")
    idxd = nc.dram_tensor("idx", [128, IDXC], I16, kind="ExternalInput")
    dstld = nc.dram_tensor("dstl", [128, TT], BF16, kind="ExternalInput")
    degd = nc.dram_tensor("degP", [128, NBLK], FP32, kind="ExternalInput")
    invdd = nc.dram_tensor("invdP", [128, NBLK], FP32, kind="ExternalInput")
    identd = nc.dram_tensor("identI", [128, 128], BF16, kind="ExternalInput")
    outd = nc.dram_tensor("out", [S, DO], FP32, kind="ExternalOutput")

    with tile.TileContext(nc) as tc:
        with (
            tc.tile_pool(name="const", bufs=1) as cp,
            tc.tile_pool(name="g", bufs=6) as gp,
            tc.tile_pool(name="oh", bufs=6) as ohp,
            tc.tile_pool(name="xs", bufs=2) as xsp,
            tc.tile_pool(name="h", bufs=2) as hp,
            tc.tile_pool(name="ht", bufs=4) as htp,
            tc.tile_pool(name="ev", bufs=2) as evp,
            tc.tile_pool(name="psA", bufs=2, space="PSUM") as psap,
            tc.tile_pool(name="psT", bufs=2, space="PSUM") as pstp,
            tc.tile_pool(name="psYZ", bufs=2, space="PSUM") as psyzp,
            tc.tile_pool(name="dram", bufs=1, space="DRAM") as dp,
        ):
            def load(name, dt_, shape, src):
                t = cp.tile(shape, dt_, name=name)
                nc.sync.dma_start(out=t[:], in_=src)
                return t

            wlr0t = load("wlr0t", BF16, [128, 256], wlr0[:])
            wlr1t = load("wlr1t", BF16, [128, 256], wlr1[:])
            wlrmt = [load(f"wlrmt{i}", BF16, [128, 512], wlrm[i * 128:(i + 1) * 128, :]) for i in range(2)]
            wlrot = [load(f"wlrot{i}", BF16, [128, 128], wlro[i * 128:(i + 1) * 128, :]) for i in range(2)]
            b01_0t = load("b01_0t", BF16, [1, 256], b01_0d[:])
            b01_1t = load("b01_1t", BF16, [1, 256], b01_1d[:])
            bmt = load("bmt", BF16, [1, 512], bmd[:])
            bot = load("bot", BF16, [1, 128], bod[:])
            idxt = load("idxt", I16, [128, IDXC], idxd[:])
            dstl = load("dstlt", BF16, [128, TT], dstld[:])
            degt = load("degt", FP32, [128, NBLK], degd[:])
            invdt = load("invdt", FP32, [128, NBLK], invdd[:])

            ones_r = cp.tile([1, 128], BF16, name="ones_r")
            nc.vector.memset(ones_r[:], 1.0)

            # iota over dst cols, replicated per tile slot: [128, TPC, 128] bf16
            iota_i = cp.tile([128, TPC, 128], I16, name="iota_i")
            nc.gpsimd.iota(iota_i[:], pattern=[[0, TPC], [1, 128]], base=0,
                           channel_multiplier=0)
            iota_f = cp.tile([128, TPC, 128], BF16, name="iota_f")
            nc.vector.tensor_copy(out=iota_f[:], in_=iota_i[:])

            ident = load("ident", BF16, [128, 128], identd[:])

            # persistent activations-free state
            acc = cp.tile([128, NBLK * 256], BF16, name="acc")
            za = cp.tile([128, NBLK * 256], BF16, name="za")     # z' for consumer layer
            zb = cp.tile([128, NBLK * 256], BF16, name="zb")     # z' produced for next layer

            shared = "Shared"
            y01_own = [dp.tile([SPH, D1], BF16, name=f"y01_own{h}") for h in range(2)]
            Y01 = [dp.tile([HALF, D1], BF16, name=f"Y01{h}", addr_space=shared) for h in range(2)]
            ym_own = [dp.tile([SPH, DM], BF16, name=f"ym_own{h}") for h in range(2)]
            Ym = [dp.tile([HALF, DM], BF16, name=f"Ym{h}", addr_space=shared) for h in range(2)]
            yo_own = [dp.tile([SPH, 128], BF16, name=f"yo_own{h}") for h in range(2)]
            Yo = [dp.tile([HALF, 128], BF16, name=f"Yo{h}", addr_space=shared) for h in range(2)]

            def write_y(dsts, b, src_tile, d):
                r0 = b * 128
                if r0 + 128 <= SPH:
                    nc.sync.dma_start(out=dsts[0][r0:r0 + 128, :], in_=src_tile[:])
                elif r0 >= SPH:
                    nc.sync.dma_start(out=dsts[1][r0 - SPH:r0 - SPH + 128, :], in_=src_tile[:])
                else:
                    nlo = SPH - r0
                    nc.sync.dma_start(out=dsts[0][r0:SPH, :], in_=src_tile[0:nlo, :])
                    nc.sync.dma_start(out=dsts[1][0:128 - nlo, :], in_=src_tile[nlo:128, :])

            RG = [list(range(NCORES))]

            def ag(src, dst):
                nc.gpsimd.collective_compute(
                    "AllGather", AL.bypass, replica_groups=RG,
                    ins=[src[:]], outs=[dst[:]])

            def blk_sl(b):
                return slice(b * 128, (b + 1) * 128)

            # ================= L1 pre: y01 = [x0@Wl0 | x1@Wl1], z' ==========
            for b in range(NBLK):
                x0b = xsp.tile([128, 128], BF16, name="x0b", tag="x0b")
                nc.sync.dma_start(out=x0b[:], in_=x0T[:, blk_sl(b)])
                x1b = xsp.tile([128, 128], BF16, name="x1b", tag="x1b")
                nc.sync.dma_start(out=x1b[:], in_=x1T[:, blk_sl(b)])
                p0 = psyzp.tile([128, 256], FP32, name="p0", tag="pyz",
                                padded_shape=[128, 512])
                p1 = psyzp.tile([128, 256], FP32, name="p1", tag="pyz",
                                padded_shape=[128, 512])
                nc.tensor.matmul(p0[:], lhsT=x0b[:], rhs=wlr0t[:], start=True, stop=False)
                nc.tensor.matmul(p0[:], lhsT=ones_r[0:1, :], rhs=b01_0t[:], start=False, stop=True)
                nc.tensor.matmul(p1[:], lhsT=x1b[:], rhs=wlr1t[:], start=True, stop=False)
                nc.tensor.matmul(p1[:], lhsT=ones_r[0:1, :], rhs=b01_1t[:], start=False, stop=True)
                evy = evp.tile([128, 256], BF16, name="evy", tag="evy")
                nc.scalar.activation(evy[:, 0:128], p0[:, 0:128], AF.Copy)
                nc.scalar.activation(evy[:, 128:256], p1[:, 0:128], AF.Copy)
                nc.vector.tensor_scalar(out=za[:, b * 256: b * 256 + 128],
                                        in0=p0[:, 128:256],
                                        scalar1=degt[:, b:b + 1], scalar2=None,
                                        op0=AL.mult)
                nc.vector.tensor_scalar(out=za[:, b * 256 + 128: (b + 1) * 256],
                                        in0=p1[:, 128:256],
                                        scalar1=degt[:, b:b + 1], scalar2=None,
                                        op0=AL.mult)
                write_y(y01_own, b, evy, D1)
                if b == NBLK // 2:
                    ag(y01_own[0], Y01[0])
            ag(y01_own[1], Y01[1])

            # ================= generic aggregation layer ====================
            def agg_layer(Ytab, elem, dcols, zin, relu, wn, bn, y_next, ynw,
                          zout, last):
                """One conv layer, 2 phases (src halves) with SBUF accumulator.

                Ytab: pair of half tables; elem: gather row width; dcols: psum
                width (256 or 64); zin: z' SBUF tensor for this layer; wn/bn:
                next-layer weight tiles [wl|wr] and bias; y_next: next y table
                halves (ynw cols); zout: z' SBUF for next layer; last: output
                layer (no relu, write out rows).
                """
                gtiles, ohs = {}, {}
                qn = [0]

                def emit_call(ci):
                    if ci in gtiles:
                        return
                    cl = calls[ci]
                    h, t0, k = cl["h"], cl["t0"], cl["k"]
                    off = 0 if h == 0 else T[0]
                    icol0 = (t0 + (0 if h == 0 else T[0])) * 8
                    g = gp.tile([128, TPC, elem], BF16, name="g", tag="g")
                    nc.gpsimd.dma_gather(
                        out_ap=g[:, 0:k, :],
                        in_ap=Ytab[h][:],
                        idxs_ap=idxt[:, icol0: icol0 + k * 8],
                        num_idxs=k * 128, num_idxs_reg=k * 128,
                        elem_size=elem, queue_num=qn[0] % 4)
                    qn[0] += 1
                    gtiles[ci] = g
                    oh = ohp.tile([128, TPC, 128], BF16, name="oh", tag="oh")
                    nc.vector.tensor_tensor(
                        out=oh[:, 0:k, :], in0=iota_f[:, 0:k, :],
                        in1=dstl[:, off + t0: off + t0 + k].to_broadcast([128, k, 128]),
                        op=AL.is_equal)
                    ohs[ci] = oh

                def phase(h):
                    base = calls.index(next(c for c in calls if c["h"] == h))
                    ncalls_h = sum(1 for c in calls if c["h"] == h)
                    for b in range(NBLK):
                        tl = block_tiles[h][b]
                        hi = max(ci for ci, _, _ in tl)
                        for ci in range(base, min(hi + 3, base + ncalls_h)):
                            emit_call(ci)
                        ps = psap.tile([128, dcols], FP32, name="ps", tag="ps",
                                       padded_shape=[128, 256])
                        if h == 0:
                            nc.tensor.matmul(ps[:], lhsT=ident[:],
                                             rhs=zin[:, b * dcols:(b + 1) * dcols],
                                             start=True, stop=False)
                        for n, (ci, slot, _) in enumerate(tl):
                            fl = (h == 1 and n == 0)
                            lastmm = (h == 0 and n == len(tl) - 1)
                            nc.tensor.matmul(ps[:], lhsT=ohs[ci][:, slot, :],
                                             rhs=gtiles[ci][:, slot, 0:dcols],
                                             start=fl, stop=lastmm)
                        if h == 0:
                            nc.scalar.activation(acc[:, b * dcols:(b + 1) * dcols],
                                                 ps[:], AF.Copy)
                        else:
                            nc.tensor.matmul(ps[:], lhsT=ident[:],
                                             rhs=acc[:, b * dcols:(b + 1) * dcols],
                                             start=False, stop=True)
                            epilogue(b, ps)

                def epilogue(b, ps):
                    if last:
                        osb = evp.tile([128, DO], FP32, name="osb", tag="osb")
                        nc.scalar.activation(osb[:], ps[:], AF.Copy,
                                             scale=invdt[:, b:b + 1])
                        rows = min(128, S - b * 128)
                        nc.sync.dma_start(out=outd[b * 128: b * 128 + rows, :],
                                          in_=osb[0:rows, :])
                        return
                    hb = hp.tile([128, 256], BF16, name="hb", tag="hb")
                    nc.scalar.activation(hb[:], ps[:], AF.Relu,
                                         scale=invdt[:, b:b + 1])
                    hts = []
                    for i in range(2):
                        pt = pstp.tile([128, 128], BF16, name="pt", tag="pt")
                        nc.tensor.transpose(pt[:], hb[:, i * 128:(i + 1) * 128], ident[:])
                        ht = htp.tile([128, 128], BF16, name="ht", tag="ht")
                        nc.scalar.activation(ht[:], pt[:], AF.Copy)
                        hts.append(ht)
                    pyz = psyzp.tile([128, 2 * ynw], FP32, name="pyz", tag="pyz",
                                     padded_shape=[128, 512])
                    nc.tensor.matmul(pyz[:], lhsT=hts[0][:], rhs=wn[0][:], start=True, stop=False)
                    nc.tensor.matmul(pyz[:], lhsT=hts[1][:], rhs=wn[1][:], start=False, stop=False)
                    nc.tensor.matmul(pyz[:], lhsT=ones_r[0:1, :], rhs=bn[:], start=False, stop=True)
                    ecols = 256 if y_next is ym_own else 128
                    evn = evp.tile([128, ecols], BF16, name="evn", tag="evy",
                                   padded_shape=[128, 256])
                    nc.scalar.activation(evn[:], pyz[:, 0:ecols], AF.Copy)
                    nc.vector.tensor_scalar(out=zout[:, b * ynw:(b + 1) * ynw],
                                            in0=pyz[:, ynw:2 * ynw],
                                            scalar1=degt[:, b:b + 1], scalar2=None,
                                            op0=AL.mult)
                    write_y(y_next, b, evn, ecols)
                    if b == NBLK // 2:
                        ag(y_next[0], (Ym if y_next is ym_own else Yo)[0])
                    elif b == NBLK - 1:
                        ag(y_next[1], (Ym if y_next is ym_own else Yo)[1])

                phase(0)
                phase(1)

            # L1: consume Y01, produce Ym + z' for Lm
            agg_layer(Y01, 256, 256, za, True, wlrmt, bmt, ym_own, 256, zb, False)
            # Lm: consume Ym, produce Yo ([y3|z3] packed) + z'3
            agg_layer(Ym, 256, 256, zb, True, wlrot, bot, yo_own, 64, za, False)
            # Lo: consume Yo (gather 128-wide rows, use cols 0:64), write out
            agg_layer(Yo, 128, 64, za, False, None, None, None, 0, None, True)

    nc.finalize()
    return nc


_CACHE = {}


def _make_inmaps(inputs, pre):
    import ml_dtypes as _ml
    BF = _ml.bfloat16
    x0 = np.asarray(inputs["x0"], np.float32)
    x1 = np.asarray(inputs["x1"], np.float32)
    deg = pre["deg"]
    Wl0 = np.asarray(inputs["Wl0"], np.float32)
    Wr0 = np.asarray(inputs["Wr0"], np.float32)
    Wl1 = np.asarray(inputs["Wl1"], np.float32)
    Wr1 = np.asarray(inputs["Wr1"], np.float32)
    Wlm = np.asarray(inputs["Wlm"], np.float32)
    Wrm = np.asarray(inputs["Wrm"], np.float32)
    Wlo = np.asarray(inputs["Wlo"], np.float32)
    Wro = np.asarray(inputs["Wro"], np.float32)
    b0 = np.asarray(inputs["b0"], np.float32)
    b1 = np.asarray(inputs["b1"], np.float32)
    bm = np.asarray(inputs["bm"], np.float32)
    bo = np.asarray(inputs["bo"], np.float32)
    z128 = np.zeros(128, np.float32)
    z256 = np.zeros(256, np.float32)
    z64 = np.zeros(64, np.float32)
    in_maps = []
    for c in range(NCORES):
        degP = np.ones((128, NBLK), np.float32)
        for b in range(NBLK):
            nrows = min(128, S - b * 128)
            if nrows > 0:
                degP[0:nrows, b] = np.maximum(deg[c, b * 128: b * 128 + nrows], 1.0)
        invdP = (1.0 / degP).astype(np.float32)
        idx_img, dstl = _idx_arrays(pre, c)
        x0c = np.zeros((128, SP), BF)
        x0c[:, :S] = x0[c * S:(c + 1) * S, :].T.astype(BF)
        x1c = np.zeros((128, SP), BF)
        x1c[:, :S] = x1[c * S:(c + 1) * S, :].T.astype(BF)
        in_maps.append({
            "x0T": x0c, "x1T": x1c,
            "wlr0": np.concatenate([Wl0, Wr0], axis=1).astype(BF),
            "wlr1": np.concatenate([Wl1, Wr1], axis=1).astype(BF),
            "wlrm": np.concatenate([Wlm, Wrm], axis=1).astype(BF),
            "wlro": np.concatenate([Wlo, Wro], axis=1).astype(BF),
            "b01_0": np.concatenate([z128, b0])[None, :].astype(BF),
            "b01_1": np.concatenate([z128, b1])[None, :].astype(BF),
            "bmc": np.concatenate([z256, bm])[None, :].astype(BF),
            "boc": np.concatenate([z64, bo])[None, :].astype(BF),
            "idx": idx_img, "dstl": dstl,
            "degP": degP, "invdP": invdP,
            "identI": np.eye(128, dtype=BF),
        })
    return in_maps


def _get_program(edge_index):
    if "prog" in _CACHE:
        return _CACHE["prog"]
    pre = _preprocess(edge_index)
    calls, block_tiles, T = _build_callplan(pre["tiles_hb"])
    nc = _build_bass(pre, calls, block_tiles, T)
    _CACHE["prog"] = (nc, pre)
    return _CACHE["prog"]


LAST_EXEC_NS = None


def kernel(**inputs):
    global LAST_EXEC_NS
    _install_hooks()
    from concourse.bass_utils import run_bass_kernel_spmd

    nc, pre = _get_program(inputs["edge_index"])
    in_maps = _make_inmaps(inputs, pre)
    trace = os.environ.get("KERNEL_TRACE", "0") == "1"
    res = run_bass_kernel_spmd(nc, in_maps, list(range(NCORES)), trace=trace)
    LAST_EXEC_NS = res.exec_time_ns
    return np.concatenate([np.asarray(res.results[c]["out"]) for c in range(NCORES)], axis=0)


# revision 23
# speedup vs baseline: 1.3228x; 1.3228x over previous
"""Self-contained Trainium2 Bass kernel for 4-layer GraphSAGE (nn_LASAGE).

Strategy (v2):
  - Nodes dst-sharded across 8 cores (6250/core, padded to 6272 = 49 blocks of 128).
  - Aggregation POST-matmul: agg(x)@Wl == agg(x@Wl). Per layer each core
    computes y = h@Wl and z = (h@Wr + b)*deg for its own shard in one fused
    per-block pass, an AllGather replicates the bf16 Y table to every core's
    DRAM, and edges gather y[src] rows with dma_gather (int16 idx, table split
    in two halves so indices fit int16).
  - Big gather calls (up to 36 tiles = 4608 idxs) spanning dst blocks amortize
    the ~1us SWDGE fixed overhead per call.
  - Scatter-add via one-hot matmuls: psum[dst, f] += onehot[e, dst].T @ g[e, f]
    with the UNWEIGHTED one-hot (iota == dstcol) as the stationary operand and
    the 256-wide gathered rows moving -> one matmul per edge tile.
  - Mean normalization folds into the epilogue: out = relu((agg + z*deg)*invd)
    via scalar-engine activation with per-partition scale, where z*deg is
    injected into the psum by an identity matmul.
  - Two phases per layer (src half 0, then 1) with a bf16 SBUF accumulator in
    between, so the second half-table AllGather hides behind phase A.
  - Final layer's table rows pack [y3 | z3] into 128 bf16 cols (256B rows, the
    dma_gather minimum) so the output scatter also runs in bf16.
"""
import sys, os, types

sys.path.insert(0, "/opt/trn_rl_repo")
import numpy as np

N = 50000
E = 800000
NCORES = 8
S = N // NCORES            # 6250 real nodes per core
SP = 6272                  # padded (49 blocks of 128)
NBLK = SP // 128
SPH = SP // 2              # 3136: local-row split for the two AG half-tables
HALF = NCORES * SPH        # 25088 rows per half-table (int16-safe)
D1 = 256                   # concat(h0, h1)
DM = 256
DO = 64
TPC = 8                    # max tiles (of 128 idxs) per dma_gather call
                           # (HW-probed: 1024 idxs OK, 1280+ crashes the device
                           # -- SWDGE descriptor-ring capacity per queue)


def _install_hooks():
    """antenv.axon_hooks shim so trace=True works in this image (optional)."""
    try:
        import antenv
        if "antenv.axon_hooks" not in sys.modules:
            mod = types.ModuleType("antenv.axon_hooks")
            mod._hook = None
            mod.set_axon_ntff_profile_hook = lambda h: setattr(mod, "_hook", h)
            mod.get_axon_ntff_profile_hook = lambda: mod._hook
            sys.modules["antenv.axon_hooks"] = mod
            antenv.axon_hooks = mod
        from antenv.axon_hooks import get_axon_ntff_profile_hook, set_axon_ntff_profile_hook
        if get_axon_ntff_profile_hook() is None:
            from trn_agent_boot.trn_boot import _ntff_profile_via_ctypes
            set_axon_ntff_profile_hook(_ntff_profile_via_ctypes("/opt/axon/libaxon_pjrt.so"))
        import concourse.bass_utils as bu
        bu.upload_artifacts = lambda tmpdir: f"file://{tmpdir}"
    except Exception:
        pass


def _preprocess(edge_index):
    """Edge lists per core, grouped by (src half, dst block), padded per-tile."""
    src = np.asarray(edge_index[0], np.int64)
    dst = np.asarray(edge_index[1], np.int64)
    core = dst // S
    dl = (dst % S).astype(np.int64)
    blk = dl // 128
    col = dl % 128
    sloc = src % S
    half = (sloc >= SPH).astype(np.int64)
    grow = (src // S) * SPH + (sloc - half * SPH)   # row within its half-table

    deg = np.bincount(core * S + dl, minlength=N).reshape(NCORES, S)

    order = np.lexsort((grow, blk, half, core))
    core_s, half_s, blk_s, col_s, row_s = (core[order], half[order], blk[order],
                                           col[order], grow[order])

    key = (core_s * 2 + half_s) * NBLK + blk_s
    counts = np.bincount(key, minlength=NCORES * 2 * NBLK).reshape(NCORES, 2, NBLK)
    tiles_hb = np.ceil(counts.max(axis=0) / 128).astype(np.int64)   # [2, NBLK]
    tiles_hb = np.maximum(tiles_hb, 1)

    pad_hb = tiles_hb * 128
    tot_h = pad_hb.sum(axis=1)
    seg_off = np.zeros((2, NBLK), np.int64)
    seg_off[:, 1:] = np.cumsum(pad_hb, axis=1)[:, :-1]

    srcpad = np.zeros((NCORES, 2), dtype=object)
    colpad = np.zeros((NCORES, 2), dtype=object)
    for c in range(NCORES):
        for h in range(2):
            srcpad[c, h] = np.zeros(int(tot_h[h]), np.int64)
            colpad[c, h] = np.full(int(tot_h[h]), -1, np.int64)
    grp = key
    first = np.r_[True, grp[1:] != grp[:-1]]
    gidx = np.arange(len(grp)) - np.maximum.accumulate(np.where(first, np.arange(len(grp)), 0))
    pos = seg_off[half_s, blk_s] + gidx
    for c in range(NCORES):
        m = core_s == c
        for h in range(2):
            mh = m & (half_s == h)
            p = pos[mh]
            srcpad[c, h][p] = row_s[mh]
            colpad[c, h][p] = col_s[mh]

    return {
        "tiles_hb": tiles_hb, "seg_off": seg_off,
        "srcpad": srcpad, "colpad": colpad, "deg": deg,
    }


def _build_callplan(tiles_hb):
    """Gather call plan: per half, chunks of up to TPC tiles spanning blocks.

    Returns calls (list of dicts) and block_tiles[h][b] = [(ci, slot, dcol)].
    dcol is the global tile column (h-major) into the dstl image.
    """
    T = [int(tiles_hb[0].sum()), int(tiles_hb[1].sum())]
    cum = np.zeros((2, NBLK), np.int64)
    cum[:, 1:] = np.cumsum(tiles_hb, axis=1)[:, :-1]
    calls = []
    call_base = [0, 0]
    for h in range(2):
        call_base[h] = len(calls)
        t = 0
        while t < T[h]:
            k = min(TPC, T[h] - t)
            calls.append(dict(h=h, t0=t, k=k))
            t += k
    block_tiles = [[[] for _ in range(NBLK)] for _ in range(2)]
    for h in range(2):
        off = 0 if h == 0 else T[0]
        for b in range(NBLK):
            for t in range(int(cum[h, b]), int(cum[h, b] + tiles_hb[h, b])):
                ci = call_base[h] + t // TPC
                slot = t % TPC
                block_tiles[h][b].append((ci, slot, off + t))
    return calls, block_tiles, T


def _idx_arrays(pre, core):
    """int16 idx image [128, (T0+T1)*8] and dst-col image [128, T0+T1] bf16."""
    import ml_dtypes as _ml
    imgs, cols = [], []
    for h in range(2):
        sp = pre["srcpad"][core, h]
        cp = pre["colpad"][core, h]
        imgs.append(sp.reshape(-1, 16).T.astype(np.int16))    # [16, tot/16]
        cols.append(cp.reshape(-1, 128).T.astype(np.float32)) # [128, T_h]
    idx_img = np.hstack(imgs)
    dstl = np.hstack(cols).astype(_ml.bfloat16)
    return np.tile(idx_img, (8, 1)), dstl


def _build_bass(pre, calls, block_tiles, T):
    import concourse.bass as bass
    import concourse.bacc as bacc
    import concourse.mybir as mybir
    import concourse.tile as tile

    FP32 = mybir.dt.float32
    BF16 = mybir.dt.bfloat16
    FP8 = mybir.dt.float8e4
    I16 = mybir.dt.int16
    AL = mybir.AluOpType
    AF = mybir.ActivationFunctionType

    TT = T[0] + T[1]
    IDXC = TT * 8

    nc = bacc.Bacc("TRN2", target_bir_lowering=False, debug=False,
                   enable_asserts=False, num_devices=NCORES, num_swdge_queues=4)

    x0T = nc.dram_tensor("x0T", [128, SP], BF16, kind="ExternalInput")
    x1T = nc.dram_tensor("x1T", [128, SP], BF16, kind="ExternalInput")
    xh0d = nc.dram_tensor("XH0", [HALF, 256], BF16, kind="ExternalInput")
    xh1d = nc.dram_tensor("XH1", [HALF, 256], BF16, kind="ExternalInput")
    wlr0 = nc.dram_tensor("wlr0", [128, 256], BF16, kind="ExternalInput")
    wlr1 = nc.dram_tensor("wlr1", [128, 256], BF16, kind="ExternalInput")
    wlrm = nc.dram_tensor("wlrm", [256, 512], BF16, kind="ExternalInput")
    wlro = nc.dram_tensor("wlro", [256, 128], BF16, kind="ExternalInput")
    b01_0d = nc.dram_tensor("b01_0", [1, 256], BF16, kind="ExternalInput")
    b01_1d = nc.dram_tensor("b01_1", [1, 256], BF16, kind="ExternalInput")
    bmd = nc.dram_tensor("bmc", [1, 512], BF16, kind="ExternalInput")
    bod = nc.dram_tensor("boc", [1, 128], BF16, kind="ExternalInput")
    idxd = nc.dram_tensor("idx", [128, IDXC], I16, kind="ExternalInput")
    dstld = nc.dram_tensor("dstl", [128, TT], BF16, kind="ExternalInput")
    degd = nc.dram_tensor("degP", [128, NBLK], FP32, kind="ExternalInput")
    invdd = nc.dram_tensor("invdP", [128, NBLK], FP32, kind="ExternalInput")
    identd = nc.dram_tensor("identI", [128, 128], BF16, kind="ExternalInput")
    outd = nc.dram_tensor("out", [S, DO], FP32, kind="ExternalOutput")

    with tile.TileContext(nc) as tc:
        with (
            tc.tile_pool(name="const", bufs=1) as cp,
            tc.tile_pool(name="g", bufs=6) as gp,
            tc.tile_pool(name="oh", bufs=6) as ohp,
            tc.tile_pool(name="xs", bufs=2) as xsp,
            tc.tile_pool(name="h", bufs=2) as hp,
            tc.tile_pool(name="ht", bufs=4) as htp,
            tc.tile_pool(name="ev", bufs=2) as evp,
            tc.tile_pool(name="psA", bufs=3, space="PSUM") as psap,
            tc.tile_pool(name="psT", bufs=2, space="PSUM") as pstp,
            tc.tile_pool(name="psYZ", bufs=2, space="PSUM") as psyzp,
            tc.tile_pool(name="dram", bufs=1, space="DRAM") as dp,
        ):
            def load(name, dt_, shape, src):
                t = cp.tile(shape, dt_, name=name)
                nc.sync.dma_start(out=t[:], in_=src)
                return t

            wlr0t = load("wlr0t", BF16, [128, 256], wlr0[:])
            wlr1t = load("wlr1t", BF16, [128, 256], wlr1[:])
            wlrmt = [load(f"wlrmt{i}", BF16, [128, 512], wlrm[i * 128:(i + 1) * 128, :]) for i in range(2)]
            wlrot = [load(f"wlrot{i}", BF16, [128, 128], wlro[i * 128:(i + 1) * 128, :]) for i in range(2)]
            b01_0t = load("b01_0t", BF16, [1, 256], b01_0d[:])
            b01_1t = load("b01_1t", BF16, [1, 256], b01_1d[:])
            bmt = load("bmt", BF16, [1, 512], bmd[:])
            bot = load("bot", BF16, [1, 128], bod[:])
            idxt = load("idxt", I16, [128, IDXC], idxd[:])
            dstl = load("dstlt", BF16, [128, TT], dstld[:])
            degt = load("degt", FP32, [128, NBLK], degd[:])
            invdt = load("invdt", FP32, [128, NBLK], invdd[:])

            ones_r = cp.tile([1, 128], BF16, name="ones_r")
            nc.vector.memset(ones_r[:], 1.0)

            # iota over dst cols, replicated per tile slot: [128, TPC, 128] bf16
            iota_i = cp.tile([128, TPC, 128], I16, name="iota_i")
            nc.gpsimd.iota(iota_i[:], pattern=[[0, TPC], [1, 128]], base=0,
                           channel_multiplier=0)
            iota_f = cp.tile([128, TPC, 128], BF16, name="iota_f")
            nc.vector.tensor_copy(out=iota_f[:], in_=iota_i[:])

            ident = load("ident", BF16, [128, 128], identd[:])

            # persistent activations-free state
            acc = cp.tile([128, NBLK * 256], BF16, name="acc")
            za = cp.tile([128, NBLK * 256], BF16, name="za")     # z' for consumer layer
            zb = cp.tile([128, NBLK * 256], BF16, name="zb")     # z' produced for next layer

            shared = "Shared"
            XH = [xh0d, xh1d]
            ym_own = [dp.tile([SPH, DM], FP8, name=f"ym_own{h}") for h in range(2)]
            Ym = [dp.tile([HALF, DM], FP8, name=f"Ym{h}", addr_space=shared) for h in range(2)]
            yo_own = [dp.tile([SPH, 128], BF16, name=f"yo_own{h}") for h in range(2)]
            Yo = [dp.tile([HALF, 128], BF16, name=f"Yo{h}", addr_space=shared) for h in range(2)]

            def write_y(dsts, b, src_tile, d):
                r0 = b * 128
                if r0 + 128 <= SPH:
                    nc.sync.dma_start(out=dsts[0][r0:r0 + 128, :], in_=src_tile[:])
                elif r0 >= SPH:
                    nc.sync.dma_start(out=dsts[1][r0 - SPH:r0 - SPH + 128, :], in_=src_tile[:])
                else:
                    nlo = SPH - r0
                    nc.sync.dma_start(out=dsts[0][r0:SPH, :], in_=src_tile[0:nlo, :])
                    nc.sync.dma_start(out=dsts[1][0:128 - nlo, :], in_=src_tile[nlo:128, :])

            RG = [list(range(NCORES))]

            def ag(src, dst):
                nc.gpsimd.collective_compute(
                    "AllGather", AL.bypass, replica_groups=RG,
                    ins=[src[:]], outs=[dst[:]])

            def blk_sl(b):
                return slice(b * 128, (b + 1) * 128)

            # ============ shared helpers ====================================
            def make_emitter(Ytab, gdt, elem):
                gtiles, ohs = {}, {}
                qn = [0]
                bases = {h: next(i for i, c in enumerate(calls) if c["h"] == h)
                         for h in range(2)}
                nch = {h: sum(1 for c in calls if c["h"] == h) for h in range(2)}

                def emit_call(ci):
                    if ci in gtiles:
                        return
                    cl = calls[ci]
                    h, t0, k = cl["h"], cl["t0"], cl["k"]
                    off = 0 if h == 0 else T[0]
                    g = gp.tile([128, TPC, elem], gdt, name="g", tag="g")
                    nc.gpsimd.dma_gather(
                        out_ap=g[:, 0:k, :],
                        in_ap=Ytab[h][:],
                        idxs_ap=idxt[:, (off + t0) * 8: (off + t0 + k) * 8],
                        num_idxs=k * 128, num_idxs_reg=k * 128,
                        elem_size=elem, queue_num=qn[0] % 4)
                    qn[0] += 1
                    gtiles[ci] = g
                    oh = ohp.tile([128, TPC, 128], gdt, name="oh", tag="oh")
                    nc.vector.tensor_tensor(
                        out=oh[:, 0:k, :], in0=iota_f[:, 0:k, :],
                        in1=dstl[:, off + t0: off + t0 + k].to_broadcast([128, k, 128]),
                        op=AL.is_equal)
                    ohs[ci] = oh

                def ensure(h, tl):
                    hi = max(ci for ci, _, _ in tl)
                    for ci in range(bases[h], min(hi + 3, bases[h] + nch[h])):
                        emit_call(ci)

                return gtiles, ohs, ensure

            def transpose2(src256):
                """[128,256] bf16 node-major -> two [128,128] bf16 feat-major."""
                hts = []
                for i in range(2):
                    pt = pstp.tile([128, 128], BF16, name="pt", tag="pt")
                    nc.tensor.transpose(pt[:], src256[:, i * 128:(i + 1) * 128], ident[:])
                    ht = htp.tile([128, 128], BF16, name="ht", tag="ht")
                    nc.scalar.activation(ht[:], pt[:], AF.Copy)
                    hts.append(ht)
                return hts

            def produce_next(b, hb, wn, bn, y_next, Ynext, ecols, ynw, evdt, zout):
                """y/z production for the next layer from this block's h."""
                hts = transpose2(hb)
                pyz = psyzp.tile([128, 2 * ynw], FP32, name="pyz", tag="pyz",
                                 padded_shape=[128, 512])
                nc.tensor.matmul(pyz[:], lhsT=hts[0][:], rhs=wn[0][:], start=True, stop=False)
                nc.tensor.matmul(pyz[:], lhsT=hts[1][:], rhs=wn[1][:], start=False, stop=False)
                nc.tensor.matmul(pyz[:], lhsT=ones_r[0:1, :], rhs=bn[:], start=False, stop=True)
                evn = evp.tile([128, ecols], evdt, name="evn", tag="evy",
                               padded_shape=[128, 256])
                nc.scalar.activation(evn[:], pyz[:, 0:ecols], AF.Copy)
                nc.vector.tensor_scalar(out=zout[:, b * ynw:(b + 1) * ynw],
                                        in0=pyz[:, ynw:2 * ynw],
                                        scalar1=degt[:, b:b + 1], scalar2=None,
                                        op0=AL.mult)
                write_y(y_next, b, evn, ecols)
                if b == NBLK // 2:
                    ag(y_next[0], Ynext[0])
                elif b == NBLK - 1:
                    ag(y_next[1], Ynext[1])

            # ============ L1 z-pre: z' = (x@Wr + b)*deg =====================
            for b in range(NBLK):
                x0b = xsp.tile([128, 128], BF16, name="x0b", tag="x0b")
                nc.sync.dma_start(out=x0b[:], in_=x0T[:, blk_sl(b)])
                x1b = xsp.tile([128, 128], BF16, name="x1b", tag="x1b")
                nc.sync.dma_start(out=x1b[:], in_=x1T[:, blk_sl(b)])
                pz = psyzp.tile([128, 256], FP32, name="pz", tag="pyz",
                                padded_shape=[128, 512])
                nc.tensor.matmul(pz[:, 0:128], lhsT=x0b[:], rhs=wlr0t[:, 128:256],
                                 start=True, stop=False)
                nc.tensor.matmul(pz[:, 0:128], lhsT=ones_r[0:1, :],
                                 rhs=b01_0t[:, 128:256], start=False, stop=True)
                nc.tensor.matmul(pz[:, 128:256], lhsT=x1b[:], rhs=wlr1t[:, 128:256],
                                 start=True, stop=False)
                nc.tensor.matmul(pz[:, 128:256], lhsT=ones_r[0:1, :],
                                 rhs=b01_1t[:, 128:256], start=False, stop=True)
                nc.vector.tensor_scalar(out=za[:, b * 256:(b + 1) * 256], in0=pz[:],
                                        scalar1=degt[:, b:b + 1], scalar2=None,
                                        op0=AL.mult)

            # ============ L1: merged single pass over the X table ===========
            # raw agg of [x0|x1] rows, then out = relu(invd*(agg@blockdiag(Wl0,
            # Wl1) + z')) since row scaling commutes with the right-multiply.
            gt1, oh1, ensure1 = make_emitter(XH, BF16, 256)
            for b in range(NBLK):
                tl = block_tiles[0][b] + block_tiles[1][b]
                ensure1(0, block_tiles[0][b])
                ensure1(1, block_tiles[1][b])
                ps = psap.tile([128, 256], FP32, name="psr", tag="ps")
                for n, (ci, slot, _) in enumerate(tl):
                    nc.tensor.matmul(ps[:], lhsT=oh1[ci][:, slot, :],
                                     rhs=gt1[ci][:, slot, :],
                                     start=(n == 0), stop=(n == len(tl) - 1))
                araw = hp.tile([128, 256], BF16, name="araw", tag="ar")
                nc.scalar.activation(araw[:], ps[:], AF.Copy)
                ats = transpose2(araw)
                ps2 = psap.tile([128, 256], FP32, name="ps2", tag="ps")
                nc.tensor.matmul(ps2[:], lhsT=ident[:],
                                 rhs=za[:, b * 256:(b + 1) * 256],
                                 start=True, stop=False)
                nc.tensor.matmul(ps2[:, 0:128], lhsT=ats[0][:], rhs=wlr0t[:, 0:128],
                                 start=False, stop=True)
                nc.tensor.matmul(ps2[:, 128:256], lhsT=ats[1][:], rhs=wlr1t[:, 0:128],
                                 start=False, stop=True)
                hb = hp.tile([128, 256], BF16, name="hb", tag="hb")
                nc.scalar.activation(hb[:], ps2[:], AF.Relu,
                                     scale=invdt[:, b:b + 1])
                produce_next(b, hb, wlrmt, bmt, ym_own, Ym, 256, 256, FP8, zb)

            # ============ generic 2-phase aggregation layer =================
            def agg_layer(Ytab, gdt, elem, dcols, zin, wn, bn, y_next, Ynext,
                          ecols, ynw, evdt, zout, last):
                gtiles, ohs, ensure = make_emitter(Ytab, gdt, elem)

                def phase(h):
                    for b in range(NBLK):
                        tl = block_tiles[h][b]
                        ensure(h, tl)
                        ps = psap.tile([128, dcols], FP32, name="ps", tag="ps",
                                       padded_shape=[128, 256])
                        if h == 0:
                            nc.tensor.matmul(ps[:], lhsT=ident[:],
                                             rhs=zin[:, b * dcols:(b + 1) * dcols],
                                             start=True, stop=False)
                        for n, (ci, slot, _) in enumerate(tl):
                            fl = (h == 1 and n == 0)
                            lastmm = (h == 0 and n == len(tl) - 1)
                            nc.tensor.matmul(ps[:], lhsT=ohs[ci][:, slot, :],
                                             rhs=gtiles[ci][:, slot, 0:dcols],
                                             start=fl, stop=lastmm)
                        if h == 0:
                            nc.scalar.activation(acc[:, b * dcols:(b + 1) * dcols],
                                                 ps[:], AF.Copy)
                        else:
                            nc.tensor.matmul(ps[:], lhsT=ident[:],
                                             rhs=acc[:, b * dcols:(b + 1) * dcols],
                                             start=False, stop=True)
                            epilogue(b, ps)

                def epilogue(b, ps):
                    if last:
                        osb = evp.tile([128, DO], FP32, name="osb", tag="osb")
                        nc.scalar.activation(osb[:], ps[:], AF.Copy,
                                             scale=invdt[:, b:b + 1])
                        rows = min(128, S - b * 128)
                        nc.sync.dma_start(out=outd[b * 128: b * 128 + rows, :],
                                          in_=osb[0:rows, :])
                        return
                    hb = hp.tile([128, 256], BF16, name="hb", tag="hb")
                    nc.scalar.activation(hb[:], ps[:], AF.Relu,
                                         scale=invdt[:, b:b + 1])
                    produce_next(b, hb, wn, bn, y_next, Ynext, ecols, ynw,
                                 evdt, zout)

                phase(0)
                phase(1)

            # Lm: consume Ym (fp8), produce Yo ([y3|z3] packed bf16) + z'3
            agg_layer(Ym, FP8, 256, 256, zb, wlrot, bot, yo_own, Yo, 128, 64,
                      BF16, za, False)
            # Lo: consume Yo (gather 128-wide bf16 rows, use cols 0:64)
            agg_layer(Yo, BF16, 128, 64, za, None, None, None, None, 0, 0,
                      None, None, True)

    nc.finalize()
    return nc


_CACHE = {}


def _make_inmaps(inputs, pre):
    import ml_dtypes as _ml
    BF = _ml.bfloat16
    x0 = np.asarray(inputs["x0"], np.float32)
    x1 = np.asarray(inputs["x1"], np.float32)
    deg = pre["deg"]
    Wl0 = np.asarray(inputs["Wl0"], np.float32)
    Wr0 = np.asarray(inputs["Wr0"], np.float32)
    Wl1 = np.asarray(inputs["Wl1"], np.float32)
    Wr1 = np.asarray(inputs["Wr1"], np.float32)
    Wlm = np.asarray(inputs["Wlm"], np.float32)
    Wrm = np.asarray(inputs["Wrm"], np.float32)
    Wlo = np.asarray(inputs["Wlo"], np.float32)
    Wro = np.asarray(inputs["Wro"], np.float32)
    b0 = np.asarray(inputs["b0"], np.float32)
    b1 = np.asarray(inputs["b1"], np.float32)
    bm = np.asarray(inputs["bm"], np.float32)
    bo = np.asarray(inputs["bo"], np.float32)
    z128 = np.zeros(128, np.float32)
    z256 = np.zeros(256, np.float32)
    z64 = np.zeros(64, np.float32)
    # full [x0|x1] table in the two-half row layout used by gather indices
    XF = np.concatenate([x0, x1], axis=1)
    XH0 = np.zeros((HALF, 256), BF)
    XH1 = np.zeros((HALF, 256), BF)
    n1 = S - SPH
    for c in range(NCORES):
        XH0[c * SPH:(c + 1) * SPH] = XF[c * S: c * S + SPH].astype(BF)
        XH1[c * SPH: c * SPH + n1] = XF[c * S + SPH: (c + 1) * S].astype(BF)
    in_maps = []
    for c in range(NCORES):
        degP = np.ones((128, NBLK), np.float32)
        for b in range(NBLK):
            nrows = min(128, S - b * 128)
            if nrows > 0:
                degP[0:nrows, b] = np.maximum(deg[c, b * 128: b * 128 + nrows], 1.0)
        invdP = (1.0 / degP).astype(np.float32)
        idx_img, dstl = _idx_arrays(pre, c)
        x0c = np.zeros((128, SP), BF)
        x0c[:, :S] = x0[c * S:(c + 1) * S, :].T.astype(BF)
        x1c = np.zeros((128, SP), BF)
        x1c[:, :S] = x1[c * S:(c + 1) * S, :].T.astype(BF)
        in_maps.append({
            "x0T": x0c, "x1T": x1c,
            "wlr0": np.concatenate([Wl0, Wr0], axis=1).astype(BF),
            "wlr1": np.concatenate([Wl1, Wr1], axis=1).astype(BF),
            "wlrm": np.concatenate([Wlm, Wrm], axis=1).astype(BF),
            "wlro": np.concatenate([Wlo, Wro], axis=1).astype(BF),
            "b01_0": np.concatenate([z128, b0])[None, :].astype(BF),
            "b01_1": np.concatenate([z128, b1])[None, :].astype(BF),
            "bmc": np.concatenate([z256, bm])[None, :].astype(BF),
            "boc": np.concatenate([z64, bo])[None, :].astype(BF),
            "idx": idx_img, "dstl": dstl,
            "degP": degP, "invdP": invdP,
            "identI": np.eye(128, dtype=BF),
            "XH0": XH0, "XH1": XH1,
        })
    return in_maps


def _get_program(edge_index):
    if "prog" in _CACHE:
        return _CACHE["prog"]
    pre = _preprocess(edge_index)
    calls, block_tiles, T = _build_callplan(pre["tiles_hb"])
    nc = _build_bass(pre, calls, block_tiles, T)
    _CACHE["prog"] = (nc, pre)
    return _CACHE["prog"]


LAST_EXEC_NS = None


def kernel(**inputs):
    global LAST_EXEC_NS
    _install_hooks()
    from concourse.bass_utils import run_bass_kernel_spmd

    nc, pre = _get_program(inputs["edge_index"])
    in_maps = _make_inmaps(inputs, pre)
    trace = os.environ.get("KERNEL_TRACE", "0") == "1"
    res = run_bass_kernel_spmd(nc, in_maps, list(range(NCORES)), trace=trace)
    LAST_EXEC_NS = res.exec_time_ns
    return np.concatenate([np.asarray(res.results[c]["out"]) for c in range(NCORES)], axis=0)
